# revision 2
# baseline (speedup 1.0000x reference)
"""Trainium2 Bass kernel for nn_LogicConvUnfold.

Math: reference computes, per kernel k, windows a,b of x (gathered at
per-kernel (h,w,c) offsets) and a 16-term weighted sum of soft logic
gates over (a, b, ab).  Grouping terms by {1, a, b, ab} collapses it to

    out_k = Cab_k*a*b + Ca_k*a + Cb_k*b + C1_k

with 4 coefficients per kernel (computed on host from weights).

Sharding (8 cores): 2-way batch x 4-way kernel grid.  Core c handles
batches [4*(c%2), +4) and kernels [32*(c//2), +32).

Device layout: partition p = b_local*32 + iblk holds a 6-row halo slab
of all 8 channels of its batch: xp[b_local, :, 4*iblk : 4*iblk+6, :]
(x padded H 128->130 so the last block's halo is in bounds).  All
per-kernel window shifts (dh, dw in 0..2, channel select) then become
*free-dim* offsets, identical across partitions, so the gather is just
a dynamically-offset strided view - no data movement at all.  The view
offsets and coefficients arrive as per-core data tensors (SPMD-safe),
loaded into DVE registers / per-partition scalar operands.

Per kernel (3 DVE ops + 1 ACT op on [128 x 504] tiles):
    p   = Cab*a + Cb          (tensor_scalar, per-partition scalars)
    m   = p * b               (tensor_tensor)
    o   = Ca*a + m            (scalar_tensor_tensor)
    res = Identity(o + C1)    (activation, bias)
then one DMA of res -> out[:, kl] (OH padded 126->128; host crops).
"""

import sys

sys.path.insert(0, "/opt/trn_rl_repo")

import numpy as np

import concourse.bass as bass
import concourse.tile as tile
from concourse import bacc, mybir
from concourse.bass_utils import run_bass_kernel_spmd

B, C, H, W = 8, 8, 128, 128
K = 128
OH, OW = 126, 126
NB = 4   # batches per core
NK = 32  # kernels per core
L = 4    # output rows per block
NBLK = 32  # row blocks per batch
HP = H + 2  # padded rows
OHP = NBLK * L  # padded output rows (128)
SLAB_F = C * 6 * W  # free elems per partition in the slab (6144)


def _coeffs(weights: np.ndarray) -> np.ndarray:
    """(K,16) weights -> (K,4) [Cab, Cb, Ca, C1], computed in f64."""
    w = weights.astype(np.float64)
    cab = (w[:, 1] - w[:, 2] - w[:, 4] - 2 * w[:, 6] - w[:, 7] + w[:, 8]
           + 2 * w[:, 9] + w[:, 11] + w[:, 13] - w[:, 14])
    ca = (w[:, 2] + w[:, 3] + w[:, 6] + w[:, 7] - w[:, 8] - w[:, 9]
          - w[:, 12] - w[:, 13])
    cb = (w[:, 4] + w[:, 5] + w[:, 6] + w[:, 7] - w[:, 8] - w[:, 9]
          - w[:, 10] - w[:, 11])
    c1 = w[:, 8:16].sum(axis=1)
    return np.stack([cab, cb, ca, c1], axis=1).astype(np.float32)


def _build_program():
    nc = bacc.Bacc("TRN2", debug=False, target_bir_lowering=False)
    xp_t = nc.dram_tensor("xp", (NB, C, HP, W), mybir.dt.float32,
                          kind="ExternalInput")
    coefs_t = nc.dram_tensor("coefs", (128, 4 * NK), mybir.dt.float32,
                             kind="ExternalInput")
    offs_t = nc.dram_tensor("offs", (1, 4 * NK), mybir.dt.int32,
                            kind="ExternalInput")
    out_t = nc.dram_tensor("out", (NB, NK, OHP, OW), mybir.dt.float32,
                           kind="ExternalOutput")

    with tile.TileContext(nc) as tc:
        with (
            tc.tile_pool(name="const", bufs=1) as cpool,
            tc.tile_pool(name="work", bufs=4) as wpool,
        ):
            slab = cpool.tile([128, SLAB_F], mybir.dt.float32)
            coefs = cpool.tile([128, 4 * NK], mybir.dt.float32)
            offs = cpool.tile([1, 4 * NK], mybir.dt.int32)

            nc.sync.dma_start(out=coefs[:, :], in_=coefs_t.ap()[:, :])
            nc.sync.dma_start(out=offs[:, :], in_=offs_t.ap()[:, :])

            # Slab load: one DMA per channel.  Source rows overlap
            # (6-row halo per 4-row block) so the AP is built raw.
            slab3 = slab[:, :].rearrange("p (r w) -> p r w", w=W)
            for c in range(C):
                src = bass.AP(
                    xp_t,
                    c * HP * W,
                    [[C * HP * W, NB], [L * W, NBLK], [W, 6], [1, W]],
                )
                nc.sync.dma_start(out=slab3[:, 6 * c:6 * (c + 1), :], in_=src)

            outv = out_t.ap().rearrange("b k (ib ii) j -> b k ib ii j", ii=L)

            for kl in range(NK):
                o4 = 4 * kl
                rows = nc.values_load_multi_w_load_instructions(
                    offs[0:1, o4:o4 + 2],
                    engines=(mybir.EngineType.DVE,),
                    min_val=0, max_val=44,
                    skip_runtime_bounds_check=True,
                )[1]
                ws = nc.values_load_multi_w_load_instructions(
                    offs[0:1, o4 + 2:o4 + 4],
                    engines=(mybir.EngineType.DVE,),
                    min_val=0, max_val=2,
                    skip_runtime_bounds_check=True,
                )[1]
                a = slab3[:, bass.ds(rows[0], L), bass.ds(ws[0], OW)]
                b = slab3[:, bass.ds(rows[1], L), bass.ds(ws[1], OW)]

                pv = wpool.tile([128, L * OW], mybir.dt.float32, tag="pv")
                mv = wpool.tile([128, L * OW], mybir.dt.float32, tag="mv")
                ov = wpool.tile([128, L * OW], mybir.dt.float32, tag="ov")
                res = wpool.tile([128, L * OW], mybir.dt.float32, tag="res")
                p3 = pv[:, :].rearrange("p (i j) -> p i j", j=OW)
                m3 = mv[:, :].rearrange("p (i j) -> p i j", j=OW)
                o3 = ov[:, :].rearrange("p (i j) -> p i j", j=OW)

                nc.vector.tensor_scalar(
                    p3, a, coefs[:, o4:o4 + 1], coefs[:, o4 + 1:o4 + 2],
                    op0=mybir.AluOpType.mult, op1=mybir.AluOpType.add,
                )
                nc.vector.tensor_tensor(
                    m3, p3, b, op=mybir.AluOpType.mult,
                )
                nc.vector.scalar_tensor_tensor(
                    o3, a, coefs[:, o4 + 2:o4 + 3], m3,
                    op0=mybir.AluOpType.mult, op1=mybir.AluOpType.add,
                )
                nc.scalar.activation(
                    res[:, :], ov[:, :],
                    mybir.ActivationFunctionType.Identity,
                    bias=coefs[:, o4 + 3:o4 + 4], scale=1.0,
                )
                nc.sync.dma_start(out=outv[:, kl], in_=res[:, :])
    nc.compile()
    return nc


def _prep_inputs(x, weights, pairs_a, pairs_b):
    cf = _coeffs(np.asarray(weights))
    pa = np.asarray(pairs_a)
    pb = np.asarray(pairs_b)
    xpad = np.zeros((B, C, HP, W), dtype=np.float32)
    xpad[:, :, :H, :] = np.asarray(x)

    in_maps = []
    for core in range(8):
        bh, kq = core % 2, core // 2
        ks = slice(32 * kq, 32 * kq + 32)
        coefs = np.broadcast_to(cf[ks].reshape(1, 4 * NK),
                                (128, 4 * NK)).copy()
        offs = np.empty((1, 4 * NK), dtype=np.int32)
        for kl in range(NK):
            ha, wa, ca = pa[32 * kq + kl]
            hb, wb, cb = pb[32 * kq + kl]
            offs[0, 4 * kl + 0] = ca * 6 + ha
            offs[0, 4 * kl + 1] = cb * 6 + hb
            offs[0, 4 * kl + 2] = wa
            offs[0, 4 * kl + 3] = wb
        in_maps.append({
            "xp": np.ascontiguousarray(xpad[4 * bh:4 * bh + 4]),
            "coefs": coefs,
            "offs": offs,
        })
    return in_maps


def _assemble(results):
    full = np.empty((B, K, OH, OW), dtype=np.float32)
    for core in range(8):
        bh, kq = core % 2, core // 2
        o = results[core]["out"]
        full[4 * bh:4 * bh + 4, 32 * kq:32 * kq + 32] = o[:, :, :OH, :]
    return full


def _run(inputs, trace=False):
    nc = _build_program()
    in_maps = _prep_inputs(inputs["x"], inputs["weights"],
                           inputs["pairs_a"], inputs["pairs_b"])
    r = run_bass_kernel_spmd(nc, in_maps, core_ids=list(range(8)),
                             trace=trace)
    return _assemble(r.results), r


def kernel(**inputs) -> np.ndarray:
    out, _ = _run(inputs)
    return out


# revision 3
# speedup vs baseline: 94.7021x; 94.7021x over previous
"""Trainium2 Bass kernel for nn_LogicConvUnfold.

Math: reference computes, per kernel k, windows a,b of x (gathered at
per-kernel (h,w,c) offsets) and a 16-term weighted sum of soft logic
gates over (a, b, ab).  Grouping terms by {1, a, b, ab} collapses it to

    out_k = Cab_k*a*b + Ca_k*a + Cb_k*b + C1_k

with 4 coefficients per kernel (computed on host from weights).

Sharding (8 cores): 2-way batch x 4-way kernel grid.  Core c handles
batches [4*(c%2), +4) and kernels [32*(c//2), +32).

Device layout: partition p = b_local*32 + iblk holds a 6-row halo slab
of all 8 channels of its batch: xp[b_local, :, 4*iblk : 4*iblk+6, :]
(x padded H 128->130 so the last block's halo is in bounds).  All
per-kernel window shifts (dh, dw in 0..2, channel select) then become
*free-dim* offsets, identical across partitions, so the gather is just
a dynamically-offset strided view - no data movement at all.  The view
offsets and coefficients arrive as per-core data tensors (SPMD-safe),
loaded into DVE registers / per-partition scalar operands.

Per kernel (3 DVE ops + 1 ACT op on [128 x 504] tiles):
    p   = Cab*a + Cb          (tensor_scalar, per-partition scalars)
    m   = p * b               (tensor_tensor)
    o   = Ca*a + m            (scalar_tensor_tensor)
    res = Identity(o + C1)    (activation, bias)
then one DMA of res -> out[:, kl] (OH padded 126->128; host crops).
"""

import sys

sys.path.insert(0, "/opt/trn_rl_repo")

import numpy as np

import concourse.bass as bass
import concourse.tile as tile
from concourse import bacc, mybir
from concourse.bass_utils import run_bass_kernel_spmd

B, C, H, W = 8, 8, 128, 128
K = 128
OH, OW = 126, 126
NB = 4   # batches per core
NK = 32  # kernels per core
L = 4    # output rows per block
NBLK = 32  # row blocks per batch
HP = H + 2  # padded rows
OHP = NBLK * L  # padded output rows (128)
SLAB_F = C * 6 * W  # free elems per partition in the slab (6144)


def _coeffs(weights: np.ndarray) -> np.ndarray:
    """(K,16) weights -> (K,4) [Cab, Cb, Ca, C1], computed in f64."""
    w = weights.astype(np.float64)
    cab = (w[:, 1] - w[:, 2] - w[:, 4] - 2 * w[:, 6] - w[:, 7] + w[:, 8]
           + 2 * w[:, 9] + w[:, 11] + w[:, 13] - w[:, 14])
    ca = (w[:, 2] + w[:, 3] + w[:, 6] + w[:, 7] - w[:, 8] - w[:, 9]
          - w[:, 12] - w[:, 13])
    cb = (w[:, 4] + w[:, 5] + w[:, 6] + w[:, 7] - w[:, 8] - w[:, 9]
          - w[:, 10] - w[:, 11])
    c1 = w[:, 8:16].sum(axis=1)
    return np.stack([cab, cb, ca, c1], axis=1).astype(np.float32)


def _build_program(reps=1):
    nc = bacc.Bacc("TRN2", debug=False, target_bir_lowering=False)
    xp_t = nc.dram_tensor("xp", (NB, C, HP, W), mybir.dt.float32,
                          kind="ExternalInput")
    coefs_t = nc.dram_tensor("coefs", (128, 4 * NK), mybir.dt.float32,
                             kind="ExternalInput")
    offs_t = nc.dram_tensor("offs", (1, 4 * NK), mybir.dt.int32,
                            kind="ExternalInput")
    out_t = nc.dram_tensor("out", (NB, NK, OHP, OW), mybir.dt.float32,
                           kind="ExternalOutput")

    with tile.TileContext(nc) as tc:
        with (
            tc.tile_pool(name="const", bufs=1) as cpool,
            tc.tile_pool(name="work", bufs=4) as wpool,
        ):
          for _rep in range(reps):
            slab = cpool.tile([128, SLAB_F], mybir.dt.float32, tag="slab")
            coefs = cpool.tile([128, 4 * NK], mybir.dt.float32, tag="coefs")
            offs = cpool.tile([1, 4 * NK], mybir.dt.int32, tag="offs")

            nc.sync.dma_start(out=coefs[:, :], in_=coefs_t.ap()[:, :])
            nc.sync.dma_start(out=offs[:, :], in_=offs_t.ap()[:, :])

            # Slab load: one DMA per channel.  Source rows overlap
            # (6-row halo per 4-row block) so the AP is built raw.
            slab3 = slab[:, :].rearrange("p (r w) -> p r w", w=W)
            for c in range(C):
                src = bass.AP(
                    xp_t,
                    c * HP * W,
                    [[C * HP * W, NB], [L * W, NBLK], [W, 6], [1, W]],
                )
                nc.sync.dma_start(out=slab3[:, 6 * c:6 * (c + 1), :], in_=src)

            outv = out_t.ap().rearrange("b k (ib ii) j -> b k ib ii j", ii=L)

            for kl in range(NK):
                o4 = 4 * kl
                rows = nc.values_load_multi_w_load_instructions(
                    offs[0:1, o4:o4 + 2],
                    engines=(mybir.EngineType.DVE,),
                    min_val=0, max_val=44,
                    skip_runtime_bounds_check=True,
                )[1]
                ws = nc.values_load_multi_w_load_instructions(
                    offs[0:1, o4 + 2:o4 + 4],
                    engines=(mybir.EngineType.DVE,),
                    min_val=0, max_val=2,
                    skip_runtime_bounds_check=True,
                )[1]
                a = slab3[:, bass.ds(rows[0], L), bass.ds(ws[0], OW)]
                b = slab3[:, bass.ds(rows[1], L), bass.ds(ws[1], OW)]

                pv = wpool.tile([128, L * OW], mybir.dt.float32, tag="pv")
                mv = wpool.tile([128, L * OW], mybir.dt.float32, tag="mv")
                ov = wpool.tile([128, L * OW], mybir.dt.float32, tag="ov")
                res = wpool.tile([128, L * OW], mybir.dt.float32, tag="res")
                p3 = pv[:, :].rearrange("p (i j) -> p i j", j=OW)
                m3 = mv[:, :].rearrange("p (i j) -> p i j", j=OW)
                o3 = ov[:, :].rearrange("p (i j) -> p i j", j=OW)

                nc.vector.tensor_scalar(
                    p3, a, coefs[:, o4:o4 + 1], coefs[:, o4 + 1:o4 + 2],
                    op0=mybir.AluOpType.mult, op1=mybir.AluOpType.add,
                )
                nc.vector.tensor_tensor(
                    m3, p3, b, op=mybir.AluOpType.mult,
                )
                nc.vector.scalar_tensor_tensor(
                    o3, a, coefs[:, o4 + 2:o4 + 3], m3,
                    op0=mybir.AluOpType.mult, op1=mybir.AluOpType.add,
                )
                nc.scalar.activation(
                    res[:, :], ov[:, :],
                    mybir.ActivationFunctionType.Identity,
                    bias=coefs[:, o4 + 3:o4 + 4], scale=1.0,
                )
                nc.sync.dma_start(out=outv[:, kl], in_=res[:, :])
    nc.compile()
    return nc


def _prep_inputs(x, weights, pairs_a, pairs_b):
    cf = _coeffs(np.asarray(weights))
    pa = np.asarray(pairs_a)
    pb = np.asarray(pairs_b)
    xpad = np.zeros((B, C, HP, W), dtype=np.float32)
    xpad[:, :, :H, :] = np.asarray(x)

    in_maps = []
    for core in range(8):
        bh, kq = core % 2, core // 2
        ks = slice(32 * kq, 32 * kq + 32)
        coefs = np.broadcast_to(cf[ks].reshape(1, 4 * NK),
                                (128, 4 * NK)).copy()
        offs = np.empty((1, 4 * NK), dtype=np.int32)
        for kl in range(NK):
            ha, wa, ca = pa[32 * kq + kl]
            hb, wb, cb = pb[32 * kq + kl]
            offs[0, 4 * kl + 0] = ca * 6 + ha
            offs[0, 4 * kl + 1] = cb * 6 + hb
            offs[0, 4 * kl + 2] = wa
            offs[0, 4 * kl + 3] = wb
        in_maps.append({
            "xp": np.ascontiguousarray(xpad[4 * bh:4 * bh + 4]),
            "coefs": coefs,
            "offs": offs,
        })
    return in_maps


def _assemble(results):
    full = np.empty((B, K, OH, OW), dtype=np.float32)
    for core in range(8):
        bh, kq = core % 2, core // 2
        o = results[core]["out"]
        full[4 * bh:4 * bh + 4, 32 * kq:32 * kq + 32] = o[:, :, :OH, :]
    return full


def _run(inputs, trace=False):
    nc = _build_program()
    in_maps = _prep_inputs(inputs["x"], inputs["weights"],
                           inputs["pairs_a"], inputs["pairs_b"])
    r = run_bass_kernel_spmd(nc, in_maps, core_ids=list(range(8)),
                             trace=trace)
    return _assemble(r.results), r


def kernel(**inputs) -> np.ndarray:
    out, _ = _run(inputs)
    return out
